# revision 21
# baseline (speedup 1.0000x reference)
"""Trainium2 Bass kernel for nn_Attention (dense transformer block), v2.

Reference computation (per batch b of 2, seq N=2048, dim D=1024, 16 heads x 64):
    q = (x @ w_q) / 64                      # source double-scales by d**-0.5
    k, v = split(x @ w_kv)
    per head: out_h = softmax(causal(q_h k_h^T)) v_h
    y = concat(out) @ w_out + b_out

Sharding (8 cores): core c -> batch b = c//4, head group g = c%4 (heads 4g..4g+3).
Each core computes its 4 heads end-to-end plus its partial output projection
(rows 256g..256g+256 of w_out); the host sums the 4 partials per batch and adds
b_out.

v2 changes vs the 232us baseline:
  - all on-chip data bf16 (halves DMA + enables DVE 2x modes); y output bf16.
  - q/k projections in fp8e4 with perf_mode=DoubleRow: contraction pairs
    (d, d+128) share a PE cell -> ~1.4x on the projection matmuls. Accuracy
    cost measured in fp64 simulation: 4.3e-3 -> 5.9e-3 absmax (gate is 2e-2).
  - QK^T per head pair issued as two K=64 matmuls at row groups 0/64: the PE
    runs them concurrently (tile_position row tiling), ~2x on hardware.
  - softmax renorm: DVE reciprocal -> GpSimd partition_broadcast (PE no longer
    does the broadcast outer product); triangle mask multiply on GpSimd.
  - fine-grained head DMA (x8 in 512-col chunks) so the first projection
    starts ~2us in; chunk order 3->0 per head pair with proj/v/out-proj work
    threaded into the attention stream as per-j "fillers" to keep PE dense.
"""

from collections import deque

import numpy as np
import ml_dtypes

import concourse.bass as bass
import concourse.mybir as mybir
import concourse.tile as tile
from concourse import bacc
from concourse.bass_utils import run_bass_kernel_spmd

FP = mybir.dt.float32
BF = mybir.dt.bfloat16
F8 = mybir.dt.float8e4
EXP = mybir.ActivationFunctionType.Exp
DR = mybir.MatmulPerfMode.DoubleRow

B = 2
N = 2048  # sequence length
D = 1024  # model dim
NH = 4  # heads per core
DH = 64  # head dim
G = 256  # per-core projection width
P = 128
NT = 4  # fp8 DoubleRow k-tile pairs (4 x (2x128) = 1024)
DKT = D // P  # 8 bf16 feature k-tiles
KT = N // P  # 16 sequence k-tiles
QC = N // 512  # 4 q chunks of 512
NCORES = 8

USE_DR = False  # fp8 DoubleRow q/k projections
USE_GP_BCAST = False  # gpsimd partition_broadcast ISA op: BROKEN on HW (garbage out)
USE_GP_COPY_BCAST = True  # broadcast via gpsimd tensor_copy of a stride-0 partition AP
USE_GP_TRI = False  # triangle mask multiply on GpSimd instead of DVE


def build_bass(repeat=1, hw_loop=0, upto="full"):
    nc = bacc.Bacc("TRN2", target_bir_lowering=False, debug=False, num_devices=NCORES)

    x8 = nc.dram_tensor("x8", [P, NT, 2, N], F8, kind="ExternalInput").ap()
    w8q = nc.dram_tensor("w8q", [P, NT, 2, G], F8, kind="ExternalInput").ap()
    w8k = nc.dram_tensor("w8k", [P, NT, 2, G], F8, kind="ExternalInput").ap()
    xb = nc.dram_tensor("xb", [P, DKT, N], BF, kind="ExternalInput").ap()
    wqb = nc.dram_tensor("wqb", [P, DKT, G], BF, kind="ExternalInput").ap()
    wkb = nc.dram_tensor("wkb", [P, DKT, G], BF, kind="ExternalInput").ap()
    wv = nc.dram_tensor("wv", [P, DKT, G], BF, kind="ExternalInput").ap()
    wo = nc.dram_tensor("wo", [P, 2, D], BF, kind="ExternalInput").ap()
    tri = nc.dram_tensor("tri", [P, P], BF, kind="ExternalInput").ap()
    ones64 = nc.dram_tensor("ones64", [DH], FP, kind="ExternalInput").ap()
    y = nc.dram_tensor("y", [N, D], BF, kind="ExternalOutput").ap()

    with tile.TileContext(nc) as tc:
        with (
            tc.tile_pool(name="const", bufs=1) as const,
            tc.tile_pool(name="ptp", bufs=4) as ptp,
            tc.tile_pool(name="ysbp", bufs=2) as ysbp,
            tc.tile_pool(name="nrm", bufs=2) as nrm,
            tc.tile_pool(name="psum", bufs=2, space="PSUM") as psum,
        ):
            x8_sb = const.tile([P, NT, 2, N], F8)
            w8q_sb = const.tile([P, NT, 2, G], F8)
            w8k_sb = const.tile([P, NT, 2, G], F8)
            xb_sb = const.tile([P, DKT, N], BF)
            wqb_sb = const.tile([P, DKT, G], BF)
            wkb_sb = const.tile([P, DKT, G], BF)
            wv_sb = const.tile([P, DKT, G], BF)
            wo_sb = const.tile([P, 2, D], BF)
            tri2_sb = const.tile([P, 2, P], BF)
            qT_sb = const.tile([P, 2, N], BF)
            kT_sb = const.tile([P, 2, N], BF)
            v_sb = const.tile([P, KT, 2, 2, P], BF)
            oT_sb = const.tile([P, 2, N], BF)
            ones_col = const.tile([65, DH], FP)

            def proj8(w_sb, dst, pl, h, on_act=False):
                # fp8 DoubleRow: dst[:, pl, 512h:+512] over 4 paired k-tiles
                ps = psum.tile([P, 512], FP, tag="rb", name="ps8", bufs=2)
                for t in range(NT):
                    nc.tensor.matmul(
                        ps,
                        w_sb[:, t, :, P * pl : P * (pl + 1)],
                        x8_sb[:, t, :, 512 * h : 512 * (h + 1)],
                        start=(t == 0),
                        stop=(t == NT - 1),
                        perf_mode=DR,
                    )
                dst_sl = dst[:, pl, 512 * h : 512 * (h + 1)]
                if on_act:
                    nc.scalar.copy(dst_sl, ps)
                else:
                    nc.vector.tensor_copy(dst_sl, ps)

            def projb(w_sb, dst, pl, h, on_act=False):
                # bf16 fallback projection (8 plain k-tiles)
                ps = psum.tile([P, 512], FP, tag="rb", name="psb", bufs=2)
                for j in range(DKT):
                    nc.tensor.matmul(
                        ps,
                        w_sb[:, j, P * pl : P * (pl + 1)],
                        xb_sb[:, j, 512 * h : 512 * (h + 1)],
                        start=(j == 0),
                        stop=(j == DKT - 1),
                    )
                dst_sl = dst[:, pl, 512 * h : 512 * (h + 1)]
                if on_act:
                    nc.scalar.copy(dst_sl, ps)
                else:
                    nc.vector.tensor_copy(dst_sl, ps)

            def proj_qk(dst8, w8_sb, wb_sb, pl, h, on_act=False):
                if USE_DR:
                    proj8(w8_sb, dst8, pl, h, on_act)
                else:
                    projb(wb_sb, dst8, pl, h, on_act)

            def proj_v(jt):
                # v rows 128*jt..+128, all 4 heads at once
                ps = psum.tile([P, G], FP, tag="rb", name="ps_v", bufs=2)
                for j in range(DKT):
                    nc.tensor.matmul(
                        ps,
                        xb_sb[:, j, P * jt : P * (jt + 1)],
                        wv_sb[:, j, :],
                        start=(j == 0),
                        stop=(j == DKT - 1),
                    )
                psv = ps.rearrange("p (pr h2 d) -> p h2 pr d", pr=2, h2=2)
                nc.vector.tensor_copy(v_sb[:, jt, 0, :, 0:64], psv[:, 0])
                nc.vector.tensor_copy(v_sb[:, jt, 1, :, 64:128], psv[:, 1])

            def attn_pair(pr, c, fillers=()):
                # heads (2*pr, 2*pr+1); q columns 512*c..+512
                fillers = deque(fillers)
                outs = [
                    psum.tile([P, 512], FP, tag="out", name=f"o{h2}", bufs=2)
                    for h2 in range(2)
                ]
                last = 4 * c + 3
                pending = None  # software pipeline: AV for j-1 issues after QK_j

                def emit_av(item):
                    j, off, PT = item
                    for h2 in range(2):
                        nc.tensor.matmul(
                            outs[h2][:, off:512],
                            v_sb[:, j, h2, pr, :],
                            PT[:, 512 * h2 + off : 512 * (h2 + 1)],
                            start=(j == 0),
                            stop=(j == last),
                        )

                for j in range(4 * c + 4):
                    off = P * (j - 4 * c) if j >= 4 * c else 0
                    S = psum.tile([P, 1024], FP, tag="s", name="S")
                    for h2 in range(2):
                        base = 64 * h2
                        nc.tensor.matmul(
                            S[:, 512 * h2 + off : 512 * (h2 + 1)],
                            kT_sb[base : base + 64, pr, P * j : P * (j + 1)],
                            qT_sb[base : base + 64, pr, 512 * c + off : 512 * (c + 1)],
                        )
                    PT = ptp.tile([P, 1024], BF, tag="pt", name="PT")
                    if off == 0:
                        nc.scalar.activation(PT, S, EXP, scale=1.0 / DH)
                    else:
                        sv = S.rearrange("p (h q) -> p h q", h=2)[:, :, off:512]
                        pv = PT.rearrange("p (h q) -> p h q", h=2)[:, :, off:512]
                        nc.scalar.activation(pv, sv, EXP, scale=1.0 / DH)
                    if j >= 4 * c:  # diagonal tile: triangle mask
                        pvd = PT.rearrange("p (h q) -> p h q", h=2)[:, :, off : off + P]
                        if USE_GP_TRI:
                            nc.gpsimd.tensor_mul(pvd, pvd, tri2_sb)
                        else:
                            nc.vector.tensor_mul(pvd, pvd, tri2_sb)
                    if fillers:
                        f = fillers.popleft()
                        if f is not None:
                            f()
                    if pending is not None:
                        emit_av(pending)
                    pending = (j, off, PT)
                emit_av(pending)

                # normalize: PE already broadcast the denominator into the
                # unused 64-partition half of each outs bank (64-wide ones block
                # in v): reciprocal cross-partition-base, then aligned multiply.
                cols = slice(512 * c, 512 * (c + 1))
                r0 = nrm.tile([64, 512], FP, tag="r0", name="r0")
                nc.vector.reciprocal(r0, outs[0][64:128, :])
                nc.vector.tensor_mul(oT_sb[0:64, pr, cols], outs[0][0:64, :], r0)
                r1f = nrm.tile([P, 512], FP, tag="r1", name="r1f")
                nc.vector.reciprocal(r1f[64:128, :], outs[1][0:64, :])
                nc.vector.tensor_mul(
                    oT_sb[64:128, pr, cols], outs[1][64:128, :], r1f[64:128, :]
                )

            def out_proj(i, split=False):
                # y rows 128*i..+128; split=True puts one copy on ACT (tail only,
                # when the exp stream is finished and ACT is idle)
                ysb = ysbp.tile([P, D], BF, name="ysb")
                for n2 in range(2):
                    ps = psum.tile([P, 512], FP, tag="rb", name="ps_y", bufs=2)
                    for pr in range(2):
                        nc.tensor.matmul(
                            ps,
                            oT_sb[:, pr, P * i : P * (i + 1)],
                            wo_sb[:, pr, 512 * n2 : 512 * (n2 + 1)],
                            start=(pr == 0),
                            stop=(pr == 1),
                        )
                    ysl = ysb[:, 512 * n2 : 512 * (n2 + 1)]
                    if split and n2 == 1:
                        nc.scalar.copy(ysl, ps)
                    else:
                        nc.vector.tensor_copy(ysl, ps)
                nc.sync.dma_start(y[P * i : P * (i + 1), :], ysb)

            # ---- program ----
            def emit_program():
                # loads, dependency-first: x8 chunks gate the projections
                nc.sync.dma_start(w8k_sb, w8k)
                if USE_DR:
                    for cc in range(4):
                        nc.sync.dma_start(
                            x8_sb[:, :, :, 512 * cc : 512 * (cc + 1)],
                            x8[:, :, :, 512 * cc : 512 * (cc + 1)],
                        )
                    nc.sync.dma_start(w8q_sb, w8q)
                else:
                    nc.sync.dma_start(w8q_sb, w8q)
                    nc.sync.dma_start(wqb_sb, wqb)
                    nc.sync.dma_start(wkb_sb, wkb)
                nc.gpsimd.memset(v_sb[:, :, 0, :, 64:128], 1.0)
                nc.gpsimd.memset(v_sb[:, :, 1, :, 0:64], 1.0)
                nc.sync.dma_start(
                    tri2_sb,
                    bass.AP(tensor=tri.tensor, offset=0, ap=[[P, P], [0, 2], [1, P]]),
                )
                if not USE_GP_BCAST:
                    nc.sync.dma_start(
                        ones_col[64:65, :],
                        bass.AP(tensor=ones64.tensor, offset=0, ap=[[0, 1], [1, DH]]),
                    )
                nc.sync.dma_start(wv_sb, wv)
                for cc in range(2):
                    nc.sync.dma_start(
                        xb_sb[:, :, 1024 * cc : 1024 * (cc + 1)],
                        xb[:, :, 1024 * cc : 1024 * (cc + 1)],
                    )
                nc.sync.dma_start(wo_sb, wo)

                # head: full kT plane 0, then qT plane 0 chunk 3
                # (ping-pong PSUM tags: 'out' slots are free until attention)
                for h in range(4):
                    proj_qk(kT_sb, w8k_sb, wkb_sb, 0, h, on_act=True)
                proj_qk(qT_sb, w8q_sb, wqb_sb, 0, 3, on_act=True)
                for jt in range(3):
                    proj_v(jt)

                if upto == "proj":
                    for h in range(3):
                        proj_qk(qT_sb, w8q_sb, wqb_sb, 0, h)
                    for pl in range(2):
                        for h in range(4):
                            proj_qk(kT_sb, w8k_sb, wkb_sb, 1, h)
                            proj_qk(qT_sb, w8q_sb, wqb_sb, 1, h)
                    for jt in range(3, KT):
                        proj_v(jt)
                    return

                F = lambda fn, *a: (lambda: fn(*a))
                # pair-interleaved chunks, descending; fillers keep PE dense
                attn_pair(
                    0,
                    3,
                    [F(proj_v, jt) for jt in range(3, KT)]
                    + [
                        F(proj_qk, kT_sb, w8k_sb, wkb_sb, 1, 0),
                        F(proj_qk, kT_sb, w8k_sb, wkb_sb, 1, 1),
                        F(proj_qk, kT_sb, w8k_sb, wkb_sb, 1, 2),
                    ],
                )
                proj_qk(kT_sb, w8k_sb, wkb_sb, 1, 3, on_act=True)
                proj_qk(qT_sb, w8q_sb, wqb_sb, 1, 3, on_act=True)
                attn_pair(
                    1,
                    3,
                    [
                        F(proj_qk, qT_sb, w8q_sb, wqb_sb, 0, 2),
                        F(proj_qk, qT_sb, w8q_sb, wqb_sb, 0, 1),
                        F(proj_qk, qT_sb, w8q_sb, wqb_sb, 0, 0),
                        F(proj_qk, qT_sb, w8q_sb, wqb_sb, 1, 2),
                        F(proj_qk, qT_sb, w8q_sb, wqb_sb, 1, 1),
                        F(proj_qk, qT_sb, w8q_sb, wqb_sb, 1, 0),
                    ],
                )
                attn_pair(
                    0, 2, [None, None, None] + [F(out_proj, i) for i in (12, 13, 14, 15)]
                )
                attn_pair(1, 2, [])
                attn_pair(0, 1, [None, None] + [F(out_proj, i) for i in (8, 9, 10, 11)])
                attn_pair(1, 1, [])
                attn_pair(0, 0, [None, F(out_proj, 4), F(out_proj, 5)])
                attn_pair(1, 0, [F(out_proj, 6), F(out_proj, 7)])
                for i in range(4):
                    out_proj(i, split=True)

            if hw_loop:
                with tc.For_i(0, hw_loop, 1) as _i:
                    emit_program()
            else:
                for _rep in range(repeat):
                    emit_program()

    nc.compile()
    return nc


_NC = None


def _get_nc():
    global _NC
    if _NC is None:
        _NC = build_bass()
    return _NC


F8NP = ml_dtypes.float8_e4m3
BFNP = ml_dtypes.bfloat16


def make_in_maps(x, w_q, w_kv, w_out):
    tri = np.asarray(np.triu(np.ones((P, P), dtype=np.float32)), dtype=BFNP)
    w_q = np.asarray(w_q, dtype=np.float32)
    w_kv = np.asarray(w_kv, dtype=np.float32)
    w_out = np.asarray(w_out, dtype=np.float32)
    xTs = [np.asarray(x[b], dtype=np.float32).T for b in range(B)]
    # fp8 DoubleRow pairing: plane (t, i) holds feature rows 256t+128i+p
    x8s = [
        np.ascontiguousarray(
            xT.reshape(NT, 2, P, N).transpose(2, 0, 1, 3).astype(F8NP)
        )
        for xT in xTs
    ]
    xbs = [
        np.ascontiguousarray(xT.reshape(DKT, P, N).transpose(1, 0, 2).astype(BFNP))
        for xT in xTs
    ]
    in_maps = []
    for c in range(NCORES):
        b, g = divmod(c, NCORES // B)
        sl = slice(G * g, G * (g + 1))
        wq_g = w_q[:, sl]
        wk_g = w_kv[:, G * g : G * (g + 1)]
        wv_g = w_kv[:, D + G * g : D + G * (g + 1)]
        in_maps.append(
            {
                "x8": x8s[b],
                "w8q": np.ascontiguousarray(
                    wq_g.reshape(NT, 2, P, G).transpose(2, 0, 1, 3).astype(F8NP)
                ),
                "w8k": np.ascontiguousarray(
                    wk_g.reshape(NT, 2, P, G).transpose(2, 0, 1, 3).astype(F8NP)
                ),
                "xb": xbs[b],
                "wqb": np.ascontiguousarray(
                    wq_g.reshape(DKT, P, G).transpose(1, 0, 2).astype(BFNP)
                ),
                "wkb": np.ascontiguousarray(
                    wk_g.reshape(DKT, P, G).transpose(1, 0, 2).astype(BFNP)
                ),
                "wv": np.ascontiguousarray(
                    wv_g.reshape(DKT, P, G).transpose(1, 0, 2).astype(BFNP)
                ),
                "wo": np.ascontiguousarray(
                    w_out[sl].reshape(2, P, D).transpose(1, 0, 2).astype(BFNP)
                ),
                "tri": tri,
                "ones64": np.ones(DH, dtype=np.float32),
            }
        )
    return in_maps


def combine_outputs(results, b_out):
    b_out = np.asarray(b_out, dtype=np.float32)
    y = np.zeros((B, N, D), dtype=np.float32)
    for c in range(NCORES):
        y[c // (NCORES // B)] += np.asarray(results[c]["y"], dtype=np.float32)
    y += b_out
    return y


def kernel(x, w_q, w_kv, w_out, b_out):
    nc = _get_nc()
    in_maps = make_in_maps(x, w_q, w_kv, w_out)
    res = run_bass_kernel_spmd(nc, in_maps, core_ids=list(range(NCORES)))
    return combine_outputs(res.results, b_out)
